# revision 1
# baseline (speedup 1.0000x reference)
"""Trainium2 Bass kernel for nn_CDFL1HistogramLoss (CDF-L1 histogram loss).

Math (derived from the reference):
  1. jax.image.resize(bilinear, 512->256, antialiased) is a separable 4-tap
     filter: interior out[i] = (x[2i-1] + 3x[2i] + 3x[2i+1] + x[2i+2])/8,
     edges [3,3,1]/7.  Applied vertically via a PE matmul against a constant
     512x256 band matrix MH, horizontally via strided DVE ops.
  2. The soft histogram telescopes: with u = 256*x and c = SIGMA/256,
     cumsum(hist)[k] = T(0) - T(k+1) where T(t) = sum_x sigmoid(c*(u - t)).
  3. sigmoid saturates ~8 bins away, so each pixel only contributes
     non-trivially to a +-13-bin window around its own value.  Anchoring
     windows at 16-bin coarse buckets (h = round(u/16), w = u - 16h in
     [-8,8]) gives A[m, j] = sum_{x in bucket m} sigmoid(c*(w - t_j)) for
     t_j in [-13, 13], computable as PSUM-accumulated 128-pixel matmuls of
     (coarse one-hot) x (27 sigmoid columns + ones).  T (and the CDF
     numerators C[k]) are a fixed linear map R2 of A, precomputed on host
     (saturated tails use the exact mean of sigmoid over w~U[-8,8]).
  4. Per-channel loss = mean_k |Cp[k]/Cp[255] - Ct[k]/Ct[255]| computed
     on-device; host averages the 48 channel losses from 8 cores.

Sharding: data-parallel over batch N: core i handles batches [2i, 2i+1] of
both pred and target (12 channel-histograms, 6 pred/target pairs per core).
"""
import os
import numpy as np

import concourse.bass as bass
import concourse.bacc as bacc
import concourse.mybir as mybir
from concourse import tile
from concourse.bass_utils import run_bass_kernel_spmd

F32 = mybir.dt.float32
BF16 = mybir.dt.bfloat16
I32 = mybir.dt.int32
ALU = mybir.AluOpType
ACT = mybir.ActivationFunctionType

N_CORES = 8
BINS = 256
SIGMA = 300.0
C = SIGMA / BINS          # 1.171875
N_M = 17                  # coarse buckets h = round(u/16) in [0, 16]
T_LO, T_HI = -11, 11      # sigmoid column offsets (window +-11 fine bins)
N_T = T_HI - T_LO + 1     # 23 sigmoid columns
NT = N_T + 1              # + ones column
NPIX = 65536              # pixels per channel after resize


def make_mh() -> np.ndarray:
    """[512, 256] vertical resize matrix (jax bilinear antialiased 2x down)."""
    M = np.zeros((512, 256), dtype=np.float64)
    for i in range(256):
        if i == 0:
            M[0, 0], M[1, 0], M[2, 0] = 3 / 7, 3 / 7, 1 / 7
        elif i == 255:
            M[509, 255], M[510, 255], M[511, 255] = 1 / 7, 3 / 7, 3 / 7
        else:
            M[2 * i - 1, i] = 1 / 8
            M[2 * i, i] = 3 / 8
            M[2 * i + 1, i] = 3 / 8
            M[2 * i + 2, i] = 1 / 8
    return M.astype(np.float32)


def make_r2() -> np.ndarray:
    """R2[m, j, k]: maps A[m, j] -> C[k] = (T(0) - T(k+1))/NPIX, k = 0..255.

    A columns j = 0..N_T-1 are sigmoid(c*(w - t_j)), t_j = T_LO + j;
    column N_T is the bucket count.  For T(tp), bucket m contributes the
    exact column when trel = tp - 16m is in window, else count[m] * mean
    of sigmoid(c*(w - trel)) over w ~ U[-8, 8] (saturation tail).
    """
    def sig(z):
        return 1.0 / (1.0 + np.exp(-z))

    wq = np.linspace(-8.0, 8.0, 8193)
    Rfull = np.zeros((N_M, NT, 257))
    for m in range(N_M):
        for tp in range(257):
            trel = tp - 16 * m
            if T_LO <= trel <= T_HI:
                Rfull[m, trel - T_LO, tp] = 1.0
            else:
                Rfull[m, N_T, tp] = sig(C * (wq - trel)).mean()
    R2 = (Rfull[:, :, 0:1] - Rfull[:, :, 1:257]) / float(NPIX)
    return R2.astype(np.float32)  # [N_M, NT, 256]


def pack_r2(R2: np.ndarray) -> np.ndarray:
    """dram layout [NT, N_M*2*128]: (m, half) slice holds R2[m, :, half-k-range].T"""
    buf = np.zeros((NT, N_M * 2 * 128), dtype=np.float32)
    for m in range(N_M):
        for half in range(2):
            buf[:, (m * 2 + half) * 128:(m * 2 + half + 1) * 128] = \
                R2[m, :, half * 128:(half + 1) * 128]
    return buf


def _nonzero_blocks(MH):
    """Which (half, q) 128x128 blocks of MH are nonzero."""
    blocks = {}
    for half in range(2):
        qs = []
        for q in range(4):
            blk = MH[128 * q:128 * (q + 1), 128 * half:128 * (half + 1)]
            if np.any(blk != 0):
                qs.append(q)
        blocks[half] = qs
    return blocks


def build(n_pairs: int = 6):
    """Build the per-core Bass program. Channels: n_pairs pred + n_pairs target."""
    MH = make_mh()
    mh_blocks = _nonzero_blocks(MH)
    n_ch = 2 * n_pairs

    nc = bacc.Bacc("TRN2", target_bir_lowering=False, debug=False, num_devices=N_CORES)
    pred = nc.dram_tensor("pred", [2, 3, 512, 512], F32, kind="ExternalInput").ap()
    target = nc.dram_tensor("target", [2, 3, 512, 512], F32, kind="ExternalInput").ap()
    mh = nc.dram_tensor("mh", [512, 256], F32, kind="ExternalInput").ap()
    out = nc.dram_tensor("out", [NT, n_ch * N_M], F32, kind="ExternalOutput").ap()

    with tile.TileContext(nc) as tc:
        from contextlib import ExitStack
        nv = nc.vector
        ns = nc.scalar
        ctx = ExitStack()
        cpool = ctx.enter_context(tc.tile_pool(name="consts", bufs=1))

        # ---- constants in SBUF ----
        mh_sb = cpool.tile(shape=[128, 4, 256], dtype=F32, name="mh_sb")
        nc.sync.dma_start(mh_sb, mh.rearrange("(q p) w -> p q w", p=128))
        id_sb = cpool.tile(shape=[128, 128], dtype=F32, name="id_sb")
        idi = cpool.tile(shape=[128, 128], dtype=I32, name="idi")
        nc.gpsimd.iota(idi, pattern=[[1, 128]], base=0, channel_multiplier=-1)
        nv.tensor_scalar(id_sb, idi, 0.0, None, ALU.is_equal)
        iota_i = cpool.tile(shape=[128, 32], dtype=I32, name="iota_i")
        nc.gpsimd.iota(iota_i, pattern=[[1, 32]], base=0, channel_multiplier=0)
        iota_bf = cpool.tile(shape=[128, N_M], dtype=BF16, name="iota_bf")
        nv.tensor_copy(iota_bf, iota_i[:, 0:N_M])
        # bias[j] = -C * t_j = -C*(j + T_LO)
        bias_sb = cpool.tile(shape=[128, N_T], dtype=F32, name="bias_sb")
        nv.tensor_scalar(bias_sb, iota_i[:, 0:N_T], -C, -C * T_LO, ALU.mult, ALU.add)

        # A^T stack: partition = sigmoid-column j, free = (channel, coarse bucket m)
        a_all = cpool.tile(shape=[NT, n_ch, N_M], dtype=F32, name="a_all")

        # ---- per-channel pipeline ----
        ch_ctx = ExitStack()
        io_pool = ch_ctx.enter_context(tc.tile_pool(name="io", bufs=2))
        wk_pool = ch_ctx.enter_context(tc.tile_pool(name="wk", bufs=2))
        big_pool = ch_ctx.enter_context(tc.tile_pool(name="big", bufs=2))
        hp_pool = ch_ctx.enter_context(tc.tile_pool(name="hp", bufs=2, space="PSUM"))
        at_pool = ch_ctx.enter_context(tc.tile_pool(name="at", bufs=2, space="PSUM"))

        chans = []
        for pi in range(n_pairs):
            chans.append(("p", pi))
        for pi in range(n_pairs):
            chans.append(("t", pi))

        for ci, (grp, pi) in enumerate(chans):
            b, cch = divmod(pi, 3)
            src = (pred if grp == "p" else target)[b, cch]  # [512, 512] dram
            raw = io_pool.tile(shape=[128, 4, 512], dtype=F32, name="raw")
            nc.sync.dma_start(raw, src.rearrange("(q p) w -> p q w", p=128))

            hs = wk_pool.tile(shape=[128, 2, 512], dtype=F32, name="hs")
            up = wk_pool.tile(shape=[128, 2, 256], dtype=F32, name="up")
            a_t = wk_pool.tile(shape=[128, 2, 256], dtype=F32, name="a_t")
            q_t = wk_pool.tile(shape=[128, 2, 256], dtype=F32, name="q_t")
            qp_t = wk_pool.tile(shape=[128, 2, 256], dtype=F32, name="qp_t")

            for half in range(2):
                hp = hp_pool.tile(shape=[128, 512], dtype=F32, space="PSUM", name="hp")
                qs = mh_blocks[half]
                for qi, q in enumerate(qs):
                    nc.tensor.matmul(
                        hp, mh_sb[:, q, 128 * half:128 * (half + 1)], raw[:, q, :],
                        start=(qi == 0), stop=(qi == len(qs) - 1),
                    )
                # PSUM -> SBUF (ISA: at most one PSUM read per vector op);
                # ScalarE sits closest to PSUM.
                ns.copy(hs[:, half], hp)
                hsr = hs[:, half].rearrange("p (i two) -> p i two", two=2)
                ev, od = hsr[:, :, 0], hsr[:, :, 1]
                # A[i] = hs[2i] + hs[2i+1]; Q[i] = hs[2i+1] + hs[2i+2]
                nv.tensor_tensor(a_t[:, half], ev, od, ALU.add)
                nv.tensor_tensor(q_t[:, half, 0:255], od[:, 0:255], ev[:, 1:256], ALU.add)
                nv.tensor_tensor(qp_t[:, half, 1:255], q_t[:, half, 0:254], q_t[:, half, 1:255], ALU.add)
                # interior: u' = 2A + Qp  (= 8 * pooled value)
                nv.scalar_tensor_tensor(
                    up[:, half, 1:255], a_t[:, half, 1:255], 2.0, qp_t[:, half, 1:255],
                    ALU.mult, ALU.add,
                )
                # edges: u'[0] = (3A[0] + hs[2]) * 8/7 ; u'[255] = (3A[255] + hs[509]) * 8/7
                nv.scalar_tensor_tensor(
                    up[:, half, 0:1], a_t[:, half, 0:1], 3.0, hs[:, half, 2:3], ALU.mult, ALU.add)
                nv.tensor_scalar(up[:, half, 0:1], up[:, half, 0:1], 8.0 / 7.0, None, ALU.mult)
                nv.scalar_tensor_tensor(
                    up[:, half, 255:256], a_t[:, half, 255:256], 3.0, hs[:, half, 509:510], ALU.mult, ALU.add)
                nv.tensor_scalar(up[:, half, 255:256], up[:, half, 255:256], 8.0 / 7.0, None, ALU.mult)

            upf = up.rearrange("p h i -> p (h i)")  # [128, 512], value u/32
            # h = trunc(2u' + 0.5) = round-half-up(u/16) in [0, 16]
            h32 = wk_pool.tile(shape=[128, 512], dtype=I32, name="h32")
            nv.tensor_scalar(h32, upf, 2.0, 0.5, ALU.mult, ALU.add)
            hf = wk_pool.tile(shape=[128, 512], dtype=F32, name="hf")
            nv.tensor_copy(hf, h32)
            hbf = wk_pool.tile(shape=[128, 512], dtype=BF16, name="hbf")
            nv.tensor_copy(hbf, h32)
            # wv = 2u' - h = w/16 in [-0.5, 0.5]
            wv = wk_pool.tile(shape=[128, 512], dtype=F32, name="wv")
            nv.scalar_tensor_tensor(wv, upf, 2.0, hf, ALU.mult, ALU.subtract)
            wbf = wk_pool.tile(shape=[128, 512], dtype=BF16, name="wbf")
            ns.copy(wbf, wv)

            # sigmoid columns: sig(16C*wv - C*t_j)
            sigs = big_pool.tile(shape=[128, 512, NT], dtype=BF16, name="sigs")
            for j in range(N_T):
                ns.activation(sigs[:, :, j], wbf, ACT.Sigmoid,
                              bias=bias_sb[:, j:j + 1], scale=16.0 * C)
            nc.gpsimd.memset(sigs[:, :, N_T], 1.0)

            # coarse one-hot
            hi = big_pool.tile(shape=[128, 512, N_M], dtype=BF16, name="hi")
            nv.tensor_tensor(
                hi,
                hbf[:, :, None].broadcast_to([128, 512, N_M]),
                iota_bf[:, None, :].broadcast_to([128, 512, N_M]),
                ALU.is_equal,
            )

            # scatter: A[m, j] += onehot^T @ sigs over all 512 pixel-columns.
            # 4 pixel-columns run concurrently in separate 32-wide PE column
            # strips (tile_position), accumulating 4 partial A matrices at
            # PSUM partition offsets 0/32/64/96.
            G = 4
            # full-bank tile so partition slices land on 2KB zero-region
            # boundaries (PSUM start=True zeroes whole regions)
            a_ps4 = at_pool.tile(shape=[128, 512], dtype=F32, space="PSUM", name="a_ps4")
            for f in range(512):
                g = f % G
                nc.tensor.matmul(a_ps4[32 * g:32 * g + N_M, 0:NT], hi[:, f, :], sigs[:, f, :],
                                 start=(f < G), stop=(f >= 512 - G),
                                 tile_position=(0, 32 * g), skip_group_check=True)
            aps_sb = wk_pool.tile(shape=[128, NT], dtype=F32, name="aps_sb")
            nc.gpsimd.memset(aps_sb, 0.0)
            for g in range(G):
                nv.tensor_copy(aps_sb[32 * g:32 * g + N_M, :], a_ps4[32 * g:32 * g + N_M, 0:NT])
            at_t = at_pool.tile(shape=[NT, 128], dtype=F32, space="PSUM", name="at_t")
            nc.tensor.transpose(at_t, aps_sb, id_sb)
            att_sb = wk_pool.tile(shape=[NT, 128], dtype=F32, name="att_sb")
            ns.copy(att_sb, at_t)
            nv.tensor_tensor(a_all[:, ci, :], att_sb[:, 0:N_M], att_sb[:, 32:32 + N_M], ALU.add)
            nv.tensor_tensor(a_all[:, ci, :], a_all[:, ci, :], att_sb[:, 64:64 + N_M], ALU.add)
            nv.tensor_tensor(a_all[:, ci, :], a_all[:, ci, :], att_sb[:, 96:96 + N_M], ALU.add)

        ch_ctx.close()
        nc.sync.dma_start(out, a_all.rearrange("j c m -> j (c m)"))
        ctx.close()

    nc.compile()
    return nc


_CACHE: dict = {}
LAST_RESULT = None


def _get_nc(n_pairs=6):
    key = n_pairs
    if key not in _CACHE:
        _CACHE[key] = build(n_pairs)
    return _CACHE[key]


def kernel(pred: np.ndarray, target: np.ndarray) -> np.ndarray:
    global LAST_RESULT
    pred = np.ascontiguousarray(pred, dtype=np.float32)
    target = np.ascontiguousarray(target, dtype=np.float32)
    assert pred.shape == (16, 3, 512, 512) and target.shape == (16, 3, 512, 512)

    nc = _get_nc(6)
    mh_buf = make_mh()
    in_maps = []
    for i in range(N_CORES):
        in_maps.append({
            "pred": pred[2 * i:2 * i + 2],
            "target": target[2 * i:2 * i + 2],
            "mh": mh_buf,
        })
    trace = os.environ.get("KERNEL_TRACE", "0") == "1"
    res = run_bass_kernel_spmd(nc, in_maps, core_ids=list(range(N_CORES)), trace=trace)
    LAST_RESULT = res
    # host-side unbinning: C[k] per channel from A via R2 (float64), then loss
    R2 = make_r2().astype(np.float64).reshape(N_M * NT, 256)  # [(m, j), k]
    losses = []
    for i in range(N_CORES):
        a = res.results[i]["out"].astype(np.float64).reshape(NT, 12, N_M)
        a = a.transpose(1, 2, 0).reshape(12, N_M * NT)  # [ch, (m, j)]
        Cn = a @ R2  # [12, 256]
        for p in range(6):
            Cp, Ct = Cn[p], Cn[p + 6]
            losses.append(np.mean(np.abs(Cp / Cp[-1] - Ct / Ct[-1])))
    return np.float32(np.mean(losses))



# revision 6
# speedup vs baseline: 2.3808x; 2.3808x over previous
"""Trainium2 Bass kernel for nn_CDFL1HistogramLoss (CDF-L1 histogram loss).

Math (see reference): per channel (16->256 resized), the CDF numerators
telescope to T(t) = sum_x sig(C*(u_x - t)), C = SIGMA/256, u = 256*x.
Decompose u = 16*m + w (m = round(u/16) in [0,16], w in [-8,8]).  Each
pixel's sigmoid profile sig(C*(w - (t - 16m))) is expanded in a small
per-pixel basis (tanh anchors from ScalarE + w-powers from VectorE); the
device scatters basis columns by coarse bucket m with a PE one-hot
matmul (stationary = basis columns Phi [128, NCOL], moving = one-hot
[128, 17], PSUM-accumulated over all 512 pixel chunks in 4 PE column
strips).  The host folds A[m, col] through a least-squares fit L of
sig(C*(w - trel)) in the (bf16-quantized) device basis to get T, the
CDFs, and the loss in float64.

Sharding: data-parallel over batch: core i handles batches [2i, 2i+1]
(12 channel-histograms = 6 pred + 6 target per core).
"""
import os
import numpy as np

import concourse.bass as bass
import concourse.bacc as bacc
import concourse.mybir as mybir
from concourse import tile
from concourse.bass_utils import run_bass_kernel_spmd

F32 = mybir.dt.float32
BF16 = mybir.dt.bfloat16
I32 = mybir.dt.int32
ALU = mybir.AluOpType
ACT = mybir.ActivationFunctionType

N_CORES = 8
SIGMA = 300.0
C = SIGMA / 256.0             # z-units per fine bin (1.171875)
N_M = 17                      # coarse buckets m = round(u/16) in [0, 16]
NPIX = 65536

# ---- basis definition (device + host must agree) ----
ANCHORS = (-10.5, -7.5, -4.5, -1.5, 1.5, 4.5, 7.5, 10.5)   # fine-bin offsets
N_POW = 3                     # wn, wn^2, wn^3 with wn = w/8 in [-1, 1]
N_A = len(ANCHORS)
NCOL = N_A + N_POW + 1        # + ones column (counts); must be <= 32


def bf16_rne(x):
    f = np.asarray(x, np.float32)
    u = f.view(np.uint32)
    r = ((u >> 16) & 1) + 0x7FFF
    return ((u + r) & 0xFFFF0000).view(np.float32)


def make_mh2() -> np.ndarray:
    """[512, 256] vertical resize matrix x2 (so v = 3a + b needs no extra scale)."""
    M = np.zeros((512, 256), dtype=np.float64)
    for i in range(256):
        if i == 0:
            M[0, 0], M[1, 0], M[2, 0] = 3/7, 3/7, 1/7
        elif i == 255:
            M[509, 255], M[510, 255], M[511, 255] = 1/7, 3/7, 3/7
        else:
            M[2*i-1, i] = 1/8; M[2*i, i] = 3/8; M[2*i+1, i] = 3/8; M[2*i+2, i] = 1/8
    return (2.0 * M).astype(np.float32)


def device_basis(wv):
    """Device-accurate basis columns for wv (f32 array in [-0.5, 0.5]).

    Column order matches the Phi SBUF tile rows:
      0..N_A-1 : bf16(tanh(8C*wv - (C/2)*t_r))
      N_A + p  : wn^(p+1) chain in bf16, wn = bf16(2*wv)
      NCOL-1   : ones
    """
    wv = np.asarray(wv, np.float32)
    cols = []
    for t in ANCHORS:
        cols.append(bf16_rne(np.tanh(np.float32(8.0*C)*wv + np.float32(-(C/2.0)*t),
                                     dtype=np.float64).astype(np.float32)))
    wn = bf16_rne(2.0*wv)
    p = wn
    cols.append(p)
    for _ in range(N_POW - 1):
        p = bf16_rne(p * wn)
        cols.append(p)
    cols.append(np.ones_like(wv))
    return np.stack(cols, axis=-1)   # [..., NCOL]


def host_fit():
    """L[col, trel+256] fitting sig(C*(16*wv - trel)) in the device basis."""
    wv = np.linspace(-0.5, 0.5, 8193).astype(np.float32)
    B = device_basis(wv).astype(np.float64)
    trels = np.arange(-256, 257)
    G = 1.0/(1.0 + np.exp(-C*(16.0*wv.astype(np.float64)[:, None] - trels[None, :])))
    L, *_ = np.linalg.lstsq(B, G, rcond=None)
    return L                      # [NCOL, 513]


def build(n_pairs: int = 6):
    """Per-core Bass program: n_pairs pred + n_pairs target channels."""
    MH2 = make_mh2()
    # nonzero 128x128 blocks of MH2 per output half
    mh_blocks = {}
    for half in range(2):
        qs = []
        for q in range(4):
            if np.any(MH2[128*q:128*(q+1), 128*half:128*(half+1)] != 0):
                qs.append(q)
        mh_blocks[half] = qs
    n_ch = 2 * n_pairs

    nc = bacc.Bacc("TRN2", target_bir_lowering=False, debug=False, num_devices=N_CORES)
    pred = nc.dram_tensor("pred", [2, 3, 512, 512], F32, kind="ExternalInput").ap()
    target = nc.dram_tensor("target", [2, 3, 512, 512], F32, kind="ExternalInput").ap()
    mh = nc.dram_tensor("mh", [512, 256], F32, kind="ExternalInput").ap()
    out = nc.dram_tensor("out", [n_ch, 128, N_M], F32, kind="ExternalOutput").ap()

    with tile.TileContext(nc) as tc:
        from contextlib import ExitStack
        nv = nc.vector
        ns = nc.scalar
        ctx = ExitStack()
        cpool = ctx.enter_context(tc.tile_pool(name="consts", bufs=1))

        mh_sb = cpool.tile(shape=[128, 4, 256], dtype=F32, name="mh_sb")
        nc.sync.dma_start(mh_sb, mh.rearrange("(q p) w -> p q w", p=128))
        # bias_sb[:, r] = -(C/2)*ANCHORS[r]  (anchors uniform: -10.5 + 3r)
        iota_i = cpool.tile(shape=[128, 32], dtype=I32, name="iota_i")
        nc.gpsimd.iota(iota_i, pattern=[[1, 32]], base=0, channel_multiplier=0)
        bias_sb = cpool.tile(shape=[128, N_A], dtype=F32, name="bias_sb")
        nv.tensor_scalar(bias_sb, iota_i[:, 0:N_A], float(-3.0*(C/2.0)),
                         float(10.5*(C/2.0)), ALU.mult, ALU.add)

        ch_ctx = ExitStack()
        io_pool = ch_ctx.enter_context(tc.tile_pool(name="io", bufs=3))
        wk_pool = ch_ctx.enter_context(tc.tile_pool(name="wk", bufs=2))
        phi_pool = ch_ctx.enter_context(tc.tile_pool(name="phi", bufs=2))
        oh_pool = ch_ctx.enter_context(tc.tile_pool(name="oh", bufs=2))
        hp_pool = ch_ctx.enter_context(tc.tile_pool(name="hp", bufs=2, space="PSUM"))
        at_pool = ch_ctx.enter_context(tc.tile_pool(name="at", bufs=2, space="PSUM"))

        chans = [("p", i) for i in range(n_pairs)] + [("t", i) for i in range(n_pairs)]

        for ci, (grp, pi) in enumerate(chans):
            b, cch = divmod(pi, 3)
            src = (pred if grp == "p" else target)[b, cch]      # [512, 512] dram
            raw = io_pool.tile(shape=[128, 4, 512], dtype=F32, name="raw")
            # alternate the two HWDGE rings (sync / scalar) for input BW
            (nc.sync if ci % 2 == 0 else nc.scalar).dma_start(
                raw, src.rearrange("(q p) w -> p q w", p=128))

            hs = wk_pool.tile(shape=[128, 2, 512], dtype=F32, name="hs")
            for half in range(2):
                hp = hp_pool.tile(shape=[128, 512], dtype=F32, space="PSUM", name="hp")
                qs = mh_blocks[half]
                for qi, q in enumerate(qs):
                    nc.tensor.matmul(
                        hp, mh_sb[:, q, 128*half:128*(half+1)], raw[:, q, :],
                        start=(qi == 0), stop=(qi == len(qs) - 1),
                    )
                # PSUM -> SBUF (one per engine to balance ACT/DVE)
                if half == 0:
                    ns.copy(hs[:, half], hp)
                else:
                    nv.tensor_copy(hs[:, half], hp)

            # horizontal resize: v = 16*y in [0, 16]
            v = wk_pool.tile(shape=[128, 2, 256], dtype=F32, name="v")
            a2 = wk_pool.tile(shape=[128, 2, 256], dtype=F32, name="a2")
            b2 = wk_pool.tile(shape=[128, 2, 256], dtype=F32, name="b2")
            hsr = hs.rearrange("p h (i two) -> p h i two", two=2)
            ev, od = hsr[:, :, :, 0], hsr[:, :, :, 1]
            nv.tensor_tensor(a2, ev, od, ALU.add)                      # [128,2,256]
            nv.tensor_tensor(b2[:, :, 1:255], od[:, :, 0:254], ev[:, :, 2:256], ALU.add)
            nv.scalar_tensor_tensor(v[:, :, 1:255], a2[:, :, 1:255], 3.0,
                                    b2[:, :, 1:255], ALU.mult, ALU.add)
            for half in range(2):
                # v[0] = (8/7)*(3*a2[0] + hs[2]); v[255] = (8/7)*(3*a2[255] + hs[509])
                nv.scalar_tensor_tensor(v[:, half, 0:1], a2[:, half, 0:1], 3.0,
                                        hs[:, half, 2:3], ALU.mult, ALU.add)
                nv.tensor_scalar(v[:, half, 0:1], v[:, half, 0:1], 8.0/7.0, None, ALU.mult)
                nv.scalar_tensor_tensor(v[:, half, 255:256], a2[:, half, 255:256], 3.0,
                                        hs[:, half, 509:510], ALU.mult, ALU.add)
                nv.tensor_scalar(v[:, half, 255:256], v[:, half, 255:256], 8.0/7.0,
                                 None, ALU.mult)

            vf = v.rearrange("p h i -> p (h i)")                       # [128, 512]
            # h = round(v) in [0, 16] (DVE f32->i32 convert rounds to nearest)
            h32 = wk_pool.tile(shape=[128, 512], dtype=I32, name="h32")
            nv.tensor_copy(h32, vf)
            hbf = wk_pool.tile(shape=[128, 512], dtype=BF16, name="hbf")
            nv.tensor_copy(hbf, h32)
            hf = wk_pool.tile(shape=[128, 512], dtype=F32, name="hf")
            nv.tensor_copy(hf, h32)
            # wv = v - h in [-0.5, 0.5]
            wv = wk_pool.tile(shape=[128, 512], dtype=F32, name="wv")
            nv.tensor_tensor(wv, vf, hf, ALU.subtract)

            # basis columns Phi [128, NCOL, 512] bf16
            phi = phi_pool.tile(shape=[128, NCOL, 512], dtype=BF16, name="phi")
            for r in range(N_A):
                ns.activation(phi[:, r, :], wv, ACT.Tanh,
                              bias=bias_sb[:, r:r+1], scale=float(8.0*C))
            wn = wk_pool.tile(shape=[128, 512], dtype=BF16, name="wn")
            nv.tensor_scalar(wn, wv, 2.0, None, ALU.mult)
            nv.tensor_copy(phi[:, N_A, :], wn)
            for p in range(1, N_POW):
                nv.tensor_tensor(phi[:, N_A+p, :], phi[:, N_A+p-1, :], wn, ALU.mult)
            nv.memset(phi[:, NCOL-1, :], 1.0)

            # one-hot [128, 17, 512] bf16
            oh = oh_pool.tile(shape=[128, N_M, 512], dtype=BF16, name="oh")
            for m in range(N_M):
                nv.tensor_scalar(oh[:, m, :], hbf, float(m), None, ALU.is_equal)

            # scatter: A^T[col, m] += Phi_chunk^T @ onehot_chunk over 512 chunks,
            # 4 PE column strips (tile_position) accumulating in one PSUM bank.
            G = 4
            aps = at_pool.tile(shape=[128, 512], dtype=F32, space="PSUM", name="aps")
            for f in range(512):
                g = f % G
                nc.tensor.matmul(aps[32*g:32*g+NCOL, 0:N_M], phi[:, :, f], oh[:, :, f],
                                 start=(f < G), stop=(f >= 512 - G),
                                 tile_position=(0, 32*g), skip_group_check=True)
            stage = wk_pool.tile(shape=[128, N_M], dtype=F32, name="stage")
            for g in range(G):
                ns.copy(stage[32*g:32*g+NCOL, :], aps[32*g:32*g+NCOL, 0:N_M])
            (nc.sync if ci % 2 == 0 else nc.scalar).dma_start(out[ci], stage)

        ch_ctx.close()
        ctx.close()

    nc.compile()
    return nc


_CACHE: dict = {}
LAST_RESULT = None


def _get_nc(n_pairs=6):
    if n_pairs not in _CACHE:
        _CACHE[n_pairs] = build(n_pairs)
    return _CACHE[n_pairs]


def kernel(pred: np.ndarray, target: np.ndarray) -> np.ndarray:
    global LAST_RESULT
    pred = np.ascontiguousarray(pred, dtype=np.float32)
    target = np.ascontiguousarray(target, dtype=np.float32)
    assert pred.shape == (16, 3, 512, 512) and target.shape == (16, 3, 512, 512)

    nc = _get_nc(6)
    mh_buf = make_mh2()
    in_maps = []
    for i in range(N_CORES):
        in_maps.append({
            "pred": pred[2*i:2*i+2],
            "target": target[2*i:2*i+2],
            "mh": mh_buf,
        })
    trace = os.environ.get("KERNEL_TRACE", "0") == "1"
    res = run_bass_kernel_spmd(nc, in_maps, core_ids=list(range(N_CORES)), trace=trace)
    LAST_RESULT = res

    # host-side fold: A[col, m] -> T(tp) -> CDF -> loss (float64)
    L = host_fit()                                    # [NCOL, 513]
    tps = np.arange(257)
    losses = []
    for i in range(N_CORES):
        a = res.results[i]["out"].astype(np.float64)  # [12, 128, 17]
        A = sum(a[:, 32*s:32*s+NCOL, :] for s in range(4))   # [12, NCOL, 17]
        cdfs = []
        for ch in range(12):
            T = np.zeros(257)
            for m in range(N_M):
                T += (A[ch, :, m][:, None] * L[:, tps - 16*m + 256]).sum(axis=0)
            Cn = T[0] - T[1:]
            cdfs.append(Cn / Cn[-1])
        for p in range(6):
            losses.append(np.mean(np.abs(cdfs[p] - cdfs[p+6])))
    return np.float32(np.mean(losses))
